# revision 1
# baseline (speedup 1.0000x reference)
"""Trainium2 Bass kernel for unscaled cross-attention (key doubles as value).

Problem: B=8, Tq=Tk=2048, D=1024, fp32.
  energy = Q @ K^T  ->  softmax over Tk  ->  out = attn @ K

Sharding: batch dim across the 8 NeuronCores (1 batch element per core).

Per-core algorithm (matmuls in float32r — fp32 storage, mantissa rounded to
~12 bits, full PE rate):
  prologue: stream K in 16 row-chunks; round to f32r (K natural, resident)
            and PE-transpose to K^T [d,k] (resident).
  software-pipelined main loop over 16 q-blocks (128 rows each):
    stage A(i):  load Q block i, round to f32r, PE-transpose -> qt
    stage B(i):  S = qt.T @ K^T (PSUM [128,2048], 512-col chunks, 8 d-tile
                 accumulation per chunk; per-chunk rowmax on DVE overlaps)
                 negmax -> P = exp(S+negmax) per chunk + fused rowsum (ACT)
                 recip = 1/sum (DVE)
    stage C(i):  P^T via PE transposes; O = P^T.T @ K_nat (PSUM);
                 out = O * recip (ACT); DMA out
  Emission order per iteration: A(i+1), B(i+1) matmuls, C(i) — so stage C's
  PE work fills the wait for block i+1's softmax chain on DVE/ACT.
"""

import sys

if "/opt/trn_rl_repo" not in sys.path:
    sys.path.insert(0, "/opt/trn_rl_repo")

import numpy as np

import concourse.bacc as bacc
import concourse.tile as tile
from concourse import mybir
from concourse.bass_utils import run_bass_kernel_spmd
from concourse.masks import make_identity

N_CORES = 8
T = 2048          # Tq == Tk
D = 1024
P = 128
DO = D // P       # 8 d-tiles
KO = T // P       # 16 k-tiles
QB = T // P       # 16 q-blocks
NC4 = T // 512    # 4 S chunks
F32 = mybir.dt.float32
F32R = mybir.dt.float32r


def build_body(nc, tc, ctx, q_ap, k_ap, out_ap, n_reps=1):
    const = ctx.enter_context(tc.tile_pool(name="const", bufs=1))
    kt_pool = ctx.enter_context(tc.tile_pool(name="kt", bufs=1))
    knat_pool = ctx.enter_context(tc.tile_pool(name="knat", bufs=1))
    ld_pool = ctx.enter_context(tc.tile_pool(name="ld", bufs=2))
    qr_pool = ctx.enter_context(tc.tile_pool(name="qr", bufs=2))
    qt_pool = ctx.enter_context(tc.tile_pool(name="qt", bufs=2))
    p_pool = ctx.enter_context(tc.tile_pool(name="p", bufs=2))
    pt_pool = ctx.enter_context(tc.tile_pool(name="pt", bufs=2))
    o_pool = ctx.enter_context(tc.tile_pool(name="o", bufs=1))
    stat_pool = ctx.enter_context(tc.tile_pool(name="stat", bufs=8))
    s_psum = ctx.enter_context(tc.tile_pool(name="s_ps", bufs=1, space="PSUM"))
    tr_psum = ctx.enter_context(tc.tile_pool(name="tr_ps", bufs=2, space="PSUM"))
    o_psum = ctx.enter_context(tc.tile_pool(name="o_ps", bufs=1, space="PSUM"))

    ident = const.tile([P, P], F32)
    make_identity(nc, ident)
    ident_r = const.tile([P, P], F32R)
    nc.vector.tensor_copy(out=ident_r, in_=ident)

    kt_c = [kt_pool.tile([P, DO, 512], F32R, name=f"ktc{c}", tag=f"ktc{c}")
            for c in range(NC4)]              # kt_c[c][dd, do, kk] = K[c*512+kk, do*128+dd]
    knat = knat_pool.tile([P, KO, D], F32R)   # knat[kk, ko, d] = K[ko*128+kk, d]

    # ---- prologue: load K (1MB chunks), build knat (f32r) + kt ----
    def build_k(ko2):
        kc = ld_pool.tile([P, 2, D], F32, tag="ldk", name="kc")
        nc.sync.dma_start(
            out=kc,
            in_=k_ap[ko2 * 2 * P:(ko2 + 1) * 2 * P, :].rearrange(
                "(t p) d -> p t d", p=P),
        )
        nc.vector.tensor_copy(out=knat[:, ko2 * 2:(ko2 + 1) * 2, :], in_=kc)
        for ko in (ko2 * 2, ko2 * 2 + 1):
            for half in range(2):
                trt = tr_psum.tile([P, 4 * P], F32R, tag="tr", name="trt")
                for j in range(4):
                    do = half * 4 + j
                    nc.tensor.transpose(
                        trt[:, j * P:(j + 1) * P],
                        knat[:, ko, do * P:(do + 1) * P], ident_r
                    )
                nc.vector.tensor_copy(
                    out=kt_c[ko // 4][:, half * 4:(half + 1) * 4,
                                      (ko % 4) * P:(ko % 4 + 1) * P],
                    in_=trt.rearrange("p (j f) -> p j f", j=4),
                )

    # ---- software-pipelined main loop ----
    def stage_a(qb):
        """DMA + round + PE-transpose one Q block -> qt [d, q] tiles."""
        qc = ld_pool.tile([P, D], F32, tag="ld", name="qc")
        nc.sync.dma_start(out=qc, in_=q_ap[qb * P:(qb + 1) * P, :])
        qr = qr_pool.tile([P, D], F32R, tag="qr", name="qr")
        nc.vector.tensor_copy(out=qr, in_=qc)          # round f32 -> f32r
        qt = qt_pool.tile([P, DO, P], F32R, tag="qt", name="qt")
        for half in range(2):
            trt = tr_psum.tile([P, 4 * P], F32R, tag="tr", name="trt")
            for j in range(4):
                do = half * 4 + j
                nc.tensor.transpose(
                    trt[:, j * P:(j + 1) * P], qr[:, do * P:(do + 1) * P],
                    ident_r
                )
            nc.vector.tensor_copy(
                out=qt[:, half * 4:(half + 1) * 4, :],
                in_=trt.rearrange("p (j f) -> p j f", j=4),
            )
        return qt

    def stage_b_open():
        s_ps = s_psum.tile([P, T], F32, tag="s", name="s_ps")
        max4 = stat_pool.tile([P, NC4], F32, tag="max4", name="max4")
        return s_ps, max4

    def stage_b_chunk(s_ps, max4, qt, c4):
        for do in range(DO):
            nc.tensor.matmul(
                s_ps[:, c4 * 512:(c4 + 1) * 512],
                lhsT=qt[:, do, :],
                rhs=kt_c[c4][:, do, :],
                start=(do == 0),
                stop=(do == DO - 1),
            )
        nc.vector.tensor_reduce(
            out=max4[:, c4:c4 + 1], in_=s_ps[:, c4 * 512:(c4 + 1) * 512],
            axis=mybir.AxisListType.X, op=mybir.AluOpType.max,
        )

    def stage_b_exp(s_ps, max4):
        """negmax + chunked exp with fused row-sums (emit early: frees S)."""
        negmax = stat_pool.tile([P, 1], F32, tag="negmax", name="negmax")
        nc.vector.tensor_reduce(
            out=negmax, in_=max4, axis=mybir.AxisListType.X,
            op=mybir.AluOpType.max, negate=True,
        )
        p_sb = p_pool.tile([P, T], F32R, tag="p", name="p_sb")
        sum4 = stat_pool.tile([P, NC4], F32, tag="sum4", name="sum4")
        for c4 in range(NC4):
            nc.scalar.activation(
                out=p_sb[:, c4 * 512:(c4 + 1) * 512],
                in_=s_ps[:, c4 * 512:(c4 + 1) * 512],
                func=mybir.ActivationFunctionType.Exp,
                bias=negmax, scale=1.0,
                accum_out=sum4[:, c4:c4 + 1],
            )
        return p_sb, sum4

    def stage_b_finish(sum4):
        """sumexp + recip (emit late: keeps DVE free for trt drains)."""
        sumexp = stat_pool.tile([P, 1], F32, tag="sumexp", name="sumexp")
        nc.vector.tensor_reduce(
            out=sumexp, in_=sum4, axis=mybir.AxisListType.X,
            op=mybir.AluOpType.add,
        )
        recip = stat_pool.tile([P, 1], F32, tag="recip", name="recip")
        nc.vector.reciprocal(recip, sumexp)
        return recip

    def stage_c(qb, p_sb, recip):
        """P^T transposes + MM2 + scale + store for one q block."""
        pt = pt_pool.tile([P, KO, P], F32R, tag="pt", name="pt")
        o_ps = o_psum.tile([P, D], F32, tag="o", name="o_ps")
        for quad in range(4):
            trt = tr_psum.tile([P, 4 * P], F32R, tag="tr", name="trt")
            for j in range(4):
                ko = quad * 4 + j
                nc.tensor.transpose(
                    trt[:, j * P:(j + 1) * P], p_sb[:, ko * P:(ko + 1) * P],
                    ident_r
                )
            nc.vector.tensor_copy(
                out=pt[:, quad * 4:(quad + 1) * 4, :],
                in_=trt.rearrange("p (j f) -> p j f", j=4),
            )
        for ko in range(KO):
            for c in range(2):
                nc.tensor.matmul(
                    o_ps[:, c * 512:(c + 1) * 512],
                    lhsT=pt[:, ko, :],
                    rhs=knat[:, ko, c * 512:(c + 1) * 512],
                    start=(ko == 0),
                    stop=(ko == KO - 1),
                )
        o_sb = o_pool.tile([P, D], F32, tag="o_sb", name="o_sb")
        nc.scalar.activation(
            out=o_sb, in_=o_ps, func=mybir.ActivationFunctionType.Copy,
            scale=recip,
        )
        nc.sync.dma_start(out=out_ap[qb * P:(qb + 1) * P, :], in_=o_sb)

    for rep in range(n_reps):
        qt = stage_a(0)
        s_ps, max4 = stage_b_open()
        if rep == 0:
            # interleave K prologue with block 0's MM1 chunks: chunk c4 only
            # needs kt_c[c4] (K blocks 4*c4..4*c4+3), so MM1 fills DMA waits
            for c4 in range(NC4):
                build_k(2 * c4)
                build_k(2 * c4 + 1)
                stage_b_chunk(s_ps, max4, qt, c4)
        else:
            for c4 in range(NC4):
                stage_b_chunk(s_ps, max4, qt, c4)
        p_sb, sum4 = stage_b_exp(s_ps, max4)
        qt = stage_a(1)
        prev_p, prev_recip = p_sb, stage_b_finish(sum4)
        for qb in range(1, QB):
            s_ps, max4 = stage_b_open()
            for c4 in range(NC4):
                stage_b_chunk(s_ps, max4, qt, c4)
            p_sb, sum4 = stage_b_exp(s_ps, max4)
            if qb + 1 < QB:
                qt = stage_a(qb + 1)
            stage_c(qb - 1, prev_p, prev_recip)
            prev_p, prev_recip = p_sb, stage_b_finish(sum4)
        stage_c(QB - 1, prev_p, prev_recip)


def build_nc(n_reps=1):
    from contextlib import ExitStack

    nc = bacc.Bacc("TRN2", target_bir_lowering=False, debug=False,
                   num_devices=N_CORES)
    q_ap = nc.dram_tensor("q", [T, D], F32, kind="ExternalInput").ap()
    k_ap = nc.dram_tensor("k", [T, D], F32, kind="ExternalInput").ap()
    out_ap = nc.dram_tensor("out", [T, D], F32, kind="ExternalOutput").ap()
    with tile.TileContext(nc) as tc:
        with ExitStack() as ctx:
            build_body(nc, tc, ctx, q_ap, k_ap, out_ap, n_reps=n_reps)
    nc.compile()
    return nc


_nc_cache = {}


def kernel(query: np.ndarray, key: np.ndarray) -> np.ndarray:
    """Full unsharded inputs [8, 2048, 1024] fp32 -> output [8, 2048, 1024]."""
    assert query.shape == (N_CORES, T, D) and key.shape == (N_CORES, T, D)
    if "nc" not in _nc_cache:
        _nc_cache["nc"] = build_nc()
    nc = _nc_cache["nc"]
    in_maps = [
        {"q": np.ascontiguousarray(query[b], dtype=np.float32),
         "k": np.ascontiguousarray(key[b], dtype=np.float32)}
        for b in range(N_CORES)
    ]
    res = run_bass_kernel_spmd(nc, in_maps, list(range(N_CORES)))
    out = np.stack([res.results[b]["out"] for b in range(N_CORES)], axis=0)
    return out.astype(np.float32)

